# revision 1
# baseline (speedup 1.0000x reference)
"""Trainium2 Bass kernel for nn_CustomNLLLoss (binary-class NLL with per-class means).

Math: for C=2, the log_softmax picked value obeys
    -picked_i = softplus(x1-x0) if t=0 else softplus(x0-x1)
With d = x1 - x0, g = softplus(d) and softplus(-d) = g - d:
    sum0 = sum_{t=0} g        = S_g - S_tg
    sum1 = sum_{t=1} (g - d)  = S_tg - S_td
    loss = sum0/n0 + sum1/n1
So each core only needs S_g, S_tg, S_td, n1 over its shard, combined on host.

Per-core device work (M = 1M samples as [128 partitions x 8192]):
    POOL: d = x1 - x0 (strided sub; 2x slower than DVE but otherwise idle)
    DVE : 2 fused scalar_tensor_tensor passes (t*g, t*d) whose accum_out
          gives S_tg / S_td per partition; sub for the small tail chunks
    ACT : exp(d), ln(e+1) with accum_out => S_g; copy(t) accum => n1
    DMA : x f32 (8MiB) + targets bf16 (2MiB) in graduated chunks
All per-partition partials stream out as [P, 4, NT]; host does the final fold.
"""

import sys

for _p in ("/opt/trn_rl_repo", "/root/.axon_site/_ro/trn_rl_repo"):
    if _p not in sys.path:
        sys.path.append(_p)

import ml_dtypes
import numpy as np

import concourse.bass as bass
import concourse.tile as tile
from concourse import mybir
from concourse.bass_utils import run_bass_kernel_spmd

N_CORES = 8
N = 8388608
M = N // N_CORES      # samples per core
P = 128               # SBUF partitions
Q = M // P            # per-partition samples per core (8192)

f32 = mybir.dt.float32
bf16 = mybir.dt.bfloat16

# Graduated per-partition chunk sizes: small first chunk so compute
# starts early, shrinking tail so the serial sub->exp->ln->stt chain after
# the final DMA is short.
SIZES = [256, 1024, 1024, 1024, 1024, 1024, 1024, 640, 512, 320, 192, 128]
assert sum(SIZES) == Q
CHUNKS = []
_o = 0
for _s in SIZES:
    CHUNKS.append((_o, _s))
    _o += _s
NT = len(CHUNKS)
DVE_SUB_TAIL = 3      # this many final chunks do their sub on DVE (shorter chain)


def _legalize_waits(nc, max_waits=1):
    """This walrus build rejects instructions carrying more than ~1 sync
    wait ("Too many sync wait commands"), but Tile's Rust wait-assigner
    happily attaches several. Hoist excess waits onto same-engine NOPs
    inserted immediately before the instruction — sequencers execute waits
    in program order, so semantics are unchanged."""
    n = 0
    for f in nc.m.functions:
        for blk in f.blocks:
            il = blk.instructions
            i = 0
            while i < len(il):
                inst = il[i]
                si = getattr(inst, "sync_info", None)
                if si is not None and len(si.on_wait) > max_waits:
                    waits = list(si.on_wait)
                    extra, keep = waits[:-max_waits], waits[-max_waits:]
                    nops = []
                    for w in extra:
                        n += 1
                        nops.append(mybir.InstNoOp(
                            name=f"I-waitfix-{n}",
                            sync_info=mybir.SyncInfo(on_wait=[w], on_update=[]),
                            bass_nofuse=True,
                            engine=inst.engine,
                        ))
                    inst.sync_info = mybir.SyncInfo(
                        on_wait=keep, on_update=list(si.on_update)
                    )
                    il[i:i] = nops
                    i += len(nops)
                i += 1
    return nc


def build_nc():
    nc = bass.Bass("TRN2")
    xs = nc.declare_dram_parameter("xs", [P, Q, 2], f32, isOutput=False)
    ts = nc.declare_dram_parameter("ts", [P, Q], bf16, isOutput=False)
    # stats[:, q, i]: per-partition partial of chunk i
    # (q: 0=S_g, 1=S_tg, 2=S_td); host folds partitions and chunks.
    out = nc.declare_dram_parameter("out", [P, 3, NT], f32, isOutput=True)
    out_t = nc.declare_dram_parameter("out_t", [1, 256], f32, isOutput=True)

    with tile.TileContext(nc) as tc:
        with (
            tc.tile_pool(name="io", bufs=NT) as iop,
            tc.tile_pool(name="wk", bufs=3) as wp,
            tc.tile_pool(name="st", bufs=1) as sp,
            tc.tile_pool(name="ps", bufs=1, space="PSUM") as pp,
        ):
            stats = sp.tile([P, 3, NT], f32)
            ones = sp.tile([P, 1], bf16)
            nc.vector.memset(ones, 1.0)
            psum_t = pp.tile([1, 256], f32)
            nc.vector.memset(psum_t, 0.0)

            for i, (o0, sz) in enumerate(CHUNKS):
                xt = iop.tile([P, sz, 2], f32, tag="x")
                tt = iop.tile([P, sz], bf16, tag="t")
                nc.sync.dma_start(out=xt, in_=xs[:, o0 : o0 + sz, :])
                nc.sync.dma_start(out=tt, in_=ts[:, o0 : o0 + sz])

                # sub on POOL (otherwise idle) keeps DVE under the DMA
                # roofline; tail chunks sub on DVE for a shorter chain.
                d = wp.tile([P, sz], f32, tag="d")
                sub_eng = nc.vector if i >= NT - DVE_SUB_TAIL else nc.gpsimd
                sub_eng.tensor_tensor(
                    out=d, in0=xt[:, :, 1], in1=xt[:, :, 0],
                    op=mybir.AluOpType.subtract,
                )
                s2 = wp.tile([P, sz], f32, tag="s2")
                nc.vector.scalar_tensor_tensor(
                    out=s2, in0=tt, scalar=1.0, in1=d,
                    op0=mybir.AluOpType.mult, op1=mybir.AluOpType.mult,
                    accum_out=stats[:, 2, i : i + 1],
                )
                e = wp.tile([P, sz], f32, tag="e")
                nc.scalar.activation(
                    out=e, in_=d, func=mybir.ActivationFunctionType.Exp,
                )
                g = wp.tile([P, sz], f32, tag="g")
                nc.scalar.activation(
                    out=g, in_=e,
                    func=mybir.ActivationFunctionType.Ln,
                    bias=1.0, scale=1.0,
                    accum_out=stats[:, 0, i : i + 1],
                )
                s1 = wp.tile([P, sz], f32, tag="s1")
                nc.vector.scalar_tensor_tensor(
                    out=s1, in0=tt, scalar=1.0, in1=g,
                    op0=mybir.AluOpType.mult, op1=mybir.AluOpType.mult,
                    accum_out=stats[:, 1, i : i + 1],
                )
                # n1 partials on the otherwise-idle PE: ones^T @ t subchunks
                # accumulated into one pre-zeroed PSUM bank (start=False).
                for c0 in range(0, sz, 256):
                    cn = min(256, sz - c0)
                    nc.tensor.matmul(
                        psum_t[:, 0:cn],
                        lhsT=ones,
                        rhs=tt[:, c0 : c0 + cn],
                        start=False,
                        stop=(i == NT - 1 and c0 + 256 >= sz),
                        skip_group_check=True,
                    )

            tshow = sp.tile([1, 256], f32)
            nc.scalar.copy(out=tshow, in_=psum_t)
            nc.sync.dma_start(out=out_t[:, :], in_=tshow)
            nc.sync.dma_start(out=out[:, :, :], in_=stats)
    return _legalize_waits(nc)


_NC = None


def get_nc():
    global _NC
    if _NC is None:
        _NC = build_nc()
    return _NC


def run_device(x, tb, **spmd_kwargs):
    """x: [N,2] f32 contiguous, tb: [N] bfloat16. Returns (sums[4] float64, results)."""
    in_maps = []
    for c in range(N_CORES):
        in_maps.append({
            "xs": x[c * M : (c + 1) * M].reshape(P, Q, 2),
            "ts": tb[c * M : (c + 1) * M].reshape(P, Q),
        })
    res = run_bass_kernel_spmd(get_nc(), in_maps, list(range(N_CORES)), **spmd_kwargs)
    stats = np.stack([r["out"] for r in res.results]).astype(np.float64)
    tsum = np.stack([r["out_t"] for r in res.results]).astype(np.float64)
    s_g, s_tg, s_td = stats.sum(axis=(0, 1, 3))
    n1 = tsum.sum()
    return np.array([s_g, s_tg, s_td, n1]), res


def kernel(x, targets):
    x = np.ascontiguousarray(np.asarray(x), dtype=np.float32)
    tb = np.asarray(targets).astype(ml_dtypes.bfloat16)  # 0/1 exact in bf16
    (s_g, s_tg, s_td, n1), _ = run_device(x, tb)
    sum0 = s_g - s_tg
    sum1 = s_tg - s_td
    n0 = float(N) - n1
    p = sum0 / n0 if n0 > 0 else 0.0
    r = sum1 / n1 if n1 > 0 else 0.0
    return np.array(p + r, dtype=np.float32)



# revision 5
# speedup vs baseline: 1.2854x; 1.2854x over previous
"""Trainium2 Bass kernel for nn_CustomNLLLoss (binary-class NLL with per-class means).

Math: for C=2 with d = x1 - x0 and c = 1 - 2t in {+1, -1}:
    -picked_i = softplus(c_i * d_i)            (= softplus(d) if t=0 else softplus(-d))
So with p_i = softplus(c_i d_i):
    S_p  = sum_i p_i           = sum0 + sum1
    S_cp = sum_i c_i p_i       = sum0 - sum1
    S_c  = sum_i c_i           = n0 - n1
    loss = sum0/n0 + sum1/n1
Each core computes S_p (ACT accumulator), S_cp (PE: accumulated 128x128
c^T@p block matmuls whose PSUM diagonal holds per-column sums; host takes
the trace), and S_c (PE: ones^T @ c) over its shard; host folds.

Per-core device work (M = 1M samples as [128 partitions x 8192]):
    DMA : one packed bf16 stream per chunk, [x0 | x1 | c] segments, so
          every compute view is unit-stride (DVE 2x mode) - 6.3 MiB total
    DVE : d = x1 - x0, s = c * d      (both bf16 unit-stride, 2x mode)
    ACT : p = softplus(s), accum_out => S_p per chunk
    PE  : c^T@p diag blocks + ones^T@c, both accumulated in PSUM
All partials stream out at the end; host does the final fold.
"""

import sys

for _p in ("/opt/trn_rl_repo", "/root/.axon_site/_ro/trn_rl_repo"):
    if _p not in sys.path:
        sys.path.append(_p)

import ml_dtypes
import numpy as np

import concourse.bass as bass
import concourse.tile as tile
from concourse import mybir
from concourse.bass_utils import run_bass_kernel_spmd

N_CORES = 8
N = 8388608
M = N // N_CORES      # samples per core
P = 128               # SBUF partitions
Q = M // P            # per-partition samples per core (8192)

f32 = mybir.dt.float32
bf16 = mybir.dt.bfloat16

# Graduated per-partition chunk sizes: shrinking tail so the serial
# sub->mult->softplus->matmul chain after the final DMA is short.
SIZES = [768, 1024, 1024, 1024, 1024, 1024, 1024, 896, 256, 128]
assert sum(SIZES) == Q
CHUNKS = []
_o = 0
for _s in SIZES:
    CHUNKS.append((_o, _s))
    _o += _s
NT = len(CHUNKS)


def _legalize_waits(nc, max_waits=1):
    """This walrus build rejects instructions carrying more than ~1 sync
    wait ("Too many sync wait commands"), but Tile's Rust wait-assigner
    happily attaches several. Hoist excess waits onto same-engine NOPs
    inserted immediately before the instruction - sequencers execute waits
    in program order, so semantics are unchanged."""
    n = 0
    for f in nc.m.functions:
        for blk in f.blocks:
            il = blk.instructions
            i = 0
            while i < len(il):
                inst = il[i]
                si = getattr(inst, "sync_info", None)
                if si is not None and len(si.on_wait) > max_waits:
                    waits = list(si.on_wait)
                    extra, keep = waits[:-max_waits], waits[-max_waits:]
                    nops = []
                    for w in extra:
                        n += 1
                        nops.append(mybir.InstNoOp(
                            name=f"I-waitfix-{n}",
                            sync_info=mybir.SyncInfo(on_wait=[w], on_update=[]),
                            bass_nofuse=True,
                            engine=inst.engine,
                        ))
                    inst.sync_info = mybir.SyncInfo(
                        on_wait=keep, on_update=list(si.on_update)
                    )
                    il[i:i] = nops
                    i += len(nops)
                i += 1
    return nc


def build_nc():
    nc = bass.Bass("TRN2")
    # packed per-chunk segments [x0 (sz) | x1 (sz) | c (sz)] along the free dim
    pk = nc.declare_dram_parameter("pk", [P, 3 * Q], bf16, isOutput=False)
    out_sp = nc.declare_dram_parameter("out_sp", [P, NT], f32, isOutput=True)
    out_cp = nc.declare_dram_parameter("out_cp", [P, P], f32, isOutput=True)
    out_c = nc.declare_dram_parameter("out_c", [1, 512], f32, isOutput=True)

    n_diag = Q // P     # 64 diag-block matmuls
    n_ones = Q // 512   # 16 ones matmuls

    with tile.TileContext(nc) as tc:
        with (
            tc.tile_pool(name="io", bufs=NT) as iop,
            tc.tile_pool(name="wk", bufs=3) as wp,
            tc.tile_pool(name="st", bufs=1) as sp,
            tc.tile_pool(name="ps", bufs=1, space="PSUM") as pp,
        ):
            stats = sp.tile([P, NT], f32)
            ones = sp.tile([P, 1], bf16)
            nc.vector.memset(ones, 1.0)
            psum_cp = pp.tile([P, P], f32)
            nc.vector.memset(psum_cp, 0.0)
            psum_c = pp.tile([1, 512], f32)
            nc.vector.memset(psum_c, 0.0)

            dg = 0  # diag matmuls issued
            on = 0  # ones matmuls issued
            for i, (o0, sz) in enumerate(CHUNKS):
                ck = iop.tile([P, 3 * sz], bf16, tag="ck")
                nc.sync.dma_start(out=ck, in_=pk[:, 3 * o0 : 3 * o0 + 3 * sz])
                x0 = ck[:, 0:sz]
                x1 = ck[:, sz : 2 * sz]
                cc = ck[:, 2 * sz : 3 * sz]

                d = wp.tile([P, sz], bf16, tag="d")
                nc.vector.tensor_tensor(
                    out=d, in0=x1, in1=x0, op=mybir.AluOpType.subtract
                )
                s = wp.tile([P, sz], bf16, tag="s")
                nc.vector.tensor_tensor(
                    out=s, in0=cc, in1=d, op=mybir.AluOpType.mult
                )
                e = wp.tile([P, sz], bf16, tag="e")
                nc.scalar.activation(
                    out=e, in_=s, func=mybir.ActivationFunctionType.Exp,
                )
                p = wp.tile([P, sz], bf16, tag="p")
                nc.scalar.activation(
                    out=p, in_=e, func=mybir.ActivationFunctionType.Ln,
                    bias=1.0, scale=1.0,
                    accum_out=stats[:, i : i + 1],
                )
                # S_cp partials: c_block^T @ p_block accumulated into one
                # pre-zeroed PSUM bank; host sums the diagonal.
                for b0 in range(0, sz, P):
                    dg += 1
                    nc.tensor.matmul(
                        psum_cp,
                        lhsT=cc[:, b0 : b0 + P],
                        rhs=p[:, b0 : b0 + P],
                        start=False,
                        stop=(dg == n_diag),
                        skip_group_check=True,
                    )
                # S_c partials on the same idle PE: ones^T @ c sub-chunks
                for b0 in range(0, sz, 512):
                    bn = min(512, sz - b0)
                    on += 1
                    nc.tensor.matmul(
                        psum_c[:, 0:bn],
                        lhsT=ones,
                        rhs=cc[:, b0 : b0 + bn],
                        start=False,
                        stop=(on == n_ones),
                        skip_group_check=True,
                    )

            cp_sb = sp.tile([P, P], f32)
            nc.scalar.copy(out=cp_sb, in_=psum_cp)
            nc.sync.dma_start(out=out_cp[:, :], in_=cp_sb)
            c_sb = sp.tile([1, 512], f32)
            nc.scalar.copy(out=c_sb, in_=psum_c)
            nc.sync.dma_start(out=out_c[:, :], in_=c_sb)
            nc.sync.dma_start(out=out_sp[:, :], in_=stats)
    return _legalize_waits(nc)


_NC = None


def get_nc():
    global _NC
    if _NC is None:
        _NC = build_nc()
    return _NC


def _pack_core(xb, cb):
    """xb: [M, 2] bf16 shard, cb: [M] bf16 shard -> packed [P, 3Q] bf16."""
    xv = xb.reshape(P, Q, 2)
    cv = cb.reshape(P, Q)
    pk = np.empty((P, 3 * Q), dtype=ml_dtypes.bfloat16)
    for o0, sz in CHUNKS:
        base = 3 * o0
        pk[:, base : base + sz] = xv[:, o0 : o0 + sz, 0]
        pk[:, base + sz : base + 2 * sz] = xv[:, o0 : o0 + sz, 1]
        pk[:, base + 2 * sz : base + 3 * sz] = cv[:, o0 : o0 + sz]
    return pk


def run_device(x, targets, **spmd_kwargs):
    """x: [N,2] f32, targets: [N] int. Returns (S_p, S_cp, S_c) float64 totals."""
    xb = np.asarray(x).astype(ml_dtypes.bfloat16)
    cb = (1 - 2 * np.asarray(targets).astype(np.int8)).astype(ml_dtypes.bfloat16)
    in_maps = []
    for k in range(N_CORES):
        in_maps.append({
            "pk": _pack_core(xb[k * M : (k + 1) * M], cb[k * M : (k + 1) * M]),
        })
    res = run_bass_kernel_spmd(get_nc(), in_maps, list(range(N_CORES)), **spmd_kwargs)
    s_p = 0.0
    s_cp = 0.0
    s_c = 0.0
    for r in res.results:
        s_p += r["out_sp"].astype(np.float64).sum()
        s_cp += np.trace(r["out_cp"].astype(np.float64))
        s_c += r["out_c"].astype(np.float64).sum()
    return s_p, s_cp, s_c, res


def kernel(x, targets):
    s_p, s_cp, s_c, _ = run_device(x, targets)
    sum0 = (s_p + s_cp) / 2.0
    sum1 = (s_p - s_cp) / 2.0
    n1 = (float(N) - s_c) / 2.0
    n0 = float(N) - n1
    p = sum0 / n0 if n0 > 0 else 0.0
    r = sum1 / n1 if n1 > 0 else 0.0
    return np.array(p + r, dtype=np.float32)


# revision 14
# speedup vs baseline: 1.3486x; 1.0492x over previous
"""Trainium2 Bass kernel for nn_CustomNLLLoss (binary-class NLL with per-class means).

Math: for C=2 with d = x1 - x0 and c = 1 - 2t in {+1, -1}:
    -picked_i = softplus(c_i * d_i)            (= softplus(d) if t=0 else softplus(-d))
So with p_i = softplus(c_i d_i) = ln(1 + exp(c_i d_i)):
    S_p  = sum_i p_i           = sum0 + sum1
    S_cp = sum_i c_i p_i       = sum0 - sum1
    S_c  = sum_i c_i           = n0 - n1
    loss = sum0/n0 + sum1/n1
Per-core device work (M = 1M samples as [128 partitions x 8192], tapered chunks):
    DMA : one packed bf16 stream per chunk, [x0 | x1 | c] segments, so
          every compute view is unit-stride (DVE 2x mode) - 6.3 MiB total
    DVE : d = x1 - x0, s = c * d (bf16 unit-stride 2x); last-chunk partials
          and PSUM drains as scalar_tensor_tensor / tensor_scalar
    ACT : e = exp(s); p = ln(1 + e) (one shared ACT table set); emission is
          software-pipelined (ln lags one chunk) to avoid write-ack stalls
    PE  : S_cp via accumulated c^T @ p 128-blocks (PSUM diagonal holds
          per-column sums; host takes the trace), S_p / S_c via ones^T @ .
          All PSUM banks close at chunk NT-2 and drain under the last chunk;
          the last chunk's partials go through DVE/ACT into one tiny DMA.
Host folds the partials (the all-reduce of the sharding hint).
"""

import sys

for _p in ("/opt/trn_rl_repo", "/root/.axon_site/_ro/trn_rl_repo"):
    if _p not in sys.path:
        sys.path.append(_p)

import ml_dtypes
import numpy as np

import concourse.bass as bass
import concourse.tile as tile
from concourse import mybir
from concourse.bass_utils import run_bass_kernel_spmd

N_CORES = 8
N = 8388608
M = N // N_CORES      # samples per core
P = 128               # SBUF partitions
Q = M // P            # per-partition samples per core (8192)

f32 = mybir.dt.float32
bf16 = mybir.dt.bfloat16

SIZES = [384, 1024, 1280, 1408, 1408, 1280, 1024, 256, 128]
assert sum(SIZES) == Q
CHUNKS = []
_o = 0
for _s in SIZES:
    CHUNKS.append((_o, _s))
    _o += _s
NT = len(CHUNKS)
SZL = SIZES[-1]       # last chunk, handled off-PE


def _legalize_waits(nc, max_waits=1):
    """This walrus build rejects instructions carrying more than ~1 sync
    wait ("Too many sync wait commands"), but Tile's Rust wait-assigner
    happily attaches several. Hoist excess waits onto same-engine NOPs
    inserted immediately before the instruction - sequencers execute waits
    in program order, so semantics are unchanged."""
    n = 0
    for f in nc.m.functions:
        for blk in f.blocks:
            il = blk.instructions
            i = 0
            while i < len(il):
                inst = il[i]
                si = getattr(inst, "sync_info", None)
                if si is not None and len(si.on_wait) > max_waits:
                    waits = list(si.on_wait)
                    extra, keep = waits[:-max_waits], waits[-max_waits:]
                    nops = []
                    for w in extra:
                        n += 1
                        nops.append(mybir.InstNoOp(
                            name=f"I-waitfix-{n}",
                            sync_info=mybir.SyncInfo(on_wait=[w], on_update=[]),
                            bass_nofuse=True,
                            engine=inst.engine,
                        ))
                    inst.sync_info = mybir.SyncInfo(
                        on_wait=keep, on_update=list(si.on_update)
                    )
                    il[i:i] = nops
                    i += len(nops)
                i += 1
    return nc


def build_nc():
    nc = bass.Bass("TRN2")
    # packed per-chunk segments [x0 (sz) | x1 (sz) | c (sz)] along the free dim
    pk = nc.declare_dram_parameter("pk", [P, 3 * Q], bf16, isOutput=False)
    out_diag = nc.declare_dram_parameter("out_diag", [P, P], f32, isOutput=True)
    out_pp = nc.declare_dram_parameter("out_pp", [1, 512], f32, isOutput=True)
    out_cc = nc.declare_dram_parameter("out_cc", [1, 512], f32, isOutput=True)
    # tl cols: 0 = S_p last chunk (ACT accum), 1 = S_cp last chunk (DVE stt),
    # 2 = S_c last chunk (DVE stt with a ones tile).
    out_tl = nc.declare_dram_parameter("out_tl", [P, 3], f32, isOutput=True)

    A = mybir.ActivationFunctionType
    Op = mybir.AluOpType

    n_diag = sum(sz // P for _, sz in CHUNKS[: NT - 1])
    n_pm = sum(sz // 512 for _, sz in CHUNKS[: NT - 1])
    n_cm = n_pm

    with tile.TileContext(nc) as tc:
        with (
            tc.tile_pool(name="io", bufs=NT) as iop,
            tc.tile_pool(name="wk", bufs=4) as wp,
            tc.tile_pool(name="st", bufs=1) as sp,
            tc.tile_pool(name="ps", bufs=1, space="PSUM") as pp,
        ):
            tl = sp.tile([P, 3], f32)
            ones = sp.tile([P, 1], bf16)
            nc.vector.memset(ones, 1.0)
            ones_l = sp.tile([P, SZL], bf16)
            nc.vector.memset(ones_l, 1.0)
            psum_d = pp.tile([P, P], f32)
            nc.vector.memset(psum_d, 0.0)
            psum_p = pp.tile([1, 512], f32)
            nc.vector.memset(psum_p, 0.0)
            psum_c = pp.tile([1, 512], f32)
            nc.vector.memset(psum_c, 0.0)

            ccv = [None] * NT
            ev = [None] * NT
            pv = [None] * NT
            dg = pm = cm = 0

            def emit_ln(j):
                """ln(1+e_j) -> p_j (ACT); last chunk also accumulates S_p."""
                _, szj = CHUNKS[j]
                pj = wp.tile([P, szj], bf16, tag="p")
                pv[j] = pj
                kw = {}
                if j == NT - 1:
                    kw["accum_out"] = tl[:, 0:1]
                nc.scalar.activation(
                    out=pj, in_=ev[j], func=A.Ln, bias=1.0, scale=1.0, **kw
                )

            def emit_post(j):
                """S_cp + S_p partials for chunk j (PE early, DVE for last)."""
                nonlocal dg, pm
                _, szj = CHUNKS[j]
                if j < NT - 1:
                    for b0 in range(0, szj, P):
                        dg += 1
                        nc.tensor.matmul(
                            psum_d, lhsT=ccv[j][:, b0 : b0 + P],
                            rhs=pv[j][:, b0 : b0 + P],
                            start=False, stop=(dg == n_diag),
                            skip_group_check=True,
                        )
                    for b0 in range(0, szj, 512):
                        bn = min(512, szj - b0)
                        pm += 1
                        nc.tensor.matmul(
                            psum_p[:, 0:bn], lhsT=ones,
                            rhs=pv[j][:, b0 : b0 + bn],
                            start=False, stop=(pm == n_pm),
                            skip_group_check=True,
                        )
                else:
                    junk = wp.tile([P, szj], bf16, tag="junk")
                    nc.vector.scalar_tensor_tensor(
                        out=junk, in0=ccv[j], scalar=1.0, in1=pv[j],
                        op0=Op.mult, op1=Op.mult, accum_out=tl[:, 1:2],
                    )

            for i, (o0, sz) in enumerate(CHUNKS):
                ck = iop.tile([P, 3 * sz], bf16, tag="ck")
                nc.sync.dma_start(out=ck, in_=pk[:, 3 * o0 : 3 * o0 + 3 * sz])
                cc = ck[:, 2 * sz : 3 * sz]
                ccv[i] = cc

                d = wp.tile([P, sz], bf16, tag="d")
                nc.vector.tensor_tensor(
                    out=d, in0=ck[:, sz : 2 * sz], in1=ck[:, 0:sz], op=Op.subtract
                )
                s = wp.tile([P, sz], bf16, tag="s")
                nc.vector.tensor_tensor(out=s, in0=cc, in1=d, op=Op.mult)
                if i < NT - 1:
                    # S_c partials depend only on the DMA: keep them early on PE
                    for b0 in range(0, sz, 512):
                        bn = min(512, sz - b0)
                        cm += 1
                        nc.tensor.matmul(
                            psum_c[:, 0:bn], lhsT=ones, rhs=cc[:, b0 : b0 + bn],
                            start=False, stop=(cm == n_cm),
                            skip_group_check=True,
                        )
                else:
                    # last chunk's S_c on DVE right after its mult
                    junk2 = wp.tile([P, sz], bf16, tag="junk2")
                    nc.vector.scalar_tensor_tensor(
                        out=junk2, in0=cc, scalar=1.0, in1=ones_l,
                        op0=Op.mult, op1=Op.mult, accum_out=tl[:, 2:3],
                    )
                e = wp.tile([P, sz], bf16, tag="e")
                nc.scalar.activation(out=e, in_=s, func=A.Exp)
                ev[i] = e

                if i >= 1:
                    emit_ln(i - 1)
                    emit_post(i - 1)
                if i == NT - 1:
                    # all three PSUM banks are complete: drain them on the
                    # mostly-idle DVE while the last chunk computes
                    dg_sb = sp.tile([P, P], f32)
                    nc.vector.tensor_scalar(
                        out=dg_sb, in0=psum_d, scalar1=0.0, scalar2=None,
                        op0=Op.add,
                    )
                    nc.sync.dma_start(out=out_diag[:, :], in_=dg_sb)
                    pp_sb = sp.tile([1, 512], f32)
                    nc.vector.tensor_scalar(
                        out=pp_sb, in0=psum_p, scalar1=0.0, scalar2=None,
                        op0=Op.add,
                    )
                    nc.sync.dma_start(out=out_pp[:, :], in_=pp_sb)
                    cc_sb = sp.tile([1, 512], f32)
                    nc.vector.tensor_scalar(
                        out=cc_sb, in0=psum_c, scalar1=0.0, scalar2=None,
                        op0=Op.add,
                    )
                    nc.sync.dma_start(out=out_cc[:, :], in_=cc_sb)

            emit_ln(NT - 1)
            emit_post(NT - 1)
            nc.sync.dma_start(out=out_tl[:, :], in_=tl)
    return _legalize_waits(nc)


_NC = None


def get_nc():
    global _NC
    if _NC is None:
        _NC = build_nc()
    return _NC


def _pack_core(xb, cb):
    """xb: [M, 2] bf16 shard, cb: [M] bf16 shard -> packed [P, 3Q] bf16."""
    xv = xb.reshape(P, Q, 2)
    cv = cb.reshape(P, Q)
    pk = np.empty((P, 3 * Q), dtype=ml_dtypes.bfloat16)
    for o0, sz in CHUNKS:
        base = 3 * o0
        pk[:, base : base + sz] = xv[:, o0 : o0 + sz, 0]
        pk[:, base + sz : base + 2 * sz] = xv[:, o0 : o0 + sz, 1]
        pk[:, base + 2 * sz : base + 3 * sz] = cv[:, o0 : o0 + sz]
    return pk


def run_device(x, targets, **spmd_kwargs):
    """x: [N,2] f32, targets: [N] int. Returns (S_p, S_cp, S_c) float64 totals."""
    xb = np.asarray(x).astype(ml_dtypes.bfloat16)
    cb = (1 - 2 * np.asarray(targets).astype(np.int8)).astype(ml_dtypes.bfloat16)
    in_maps = []
    for k in range(N_CORES):
        in_maps.append({
            "pk": _pack_core(xb[k * M : (k + 1) * M], cb[k * M : (k + 1) * M]),
        })
    res = run_bass_kernel_spmd(get_nc(), in_maps, list(range(N_CORES)), **spmd_kwargs)
    s_p = 0.0
    s_cp = 0.0
    s_c = 0.0
    for r in res.results:
        tl = r["out_tl"].astype(np.float64)
        s_p += r["out_pp"].astype(np.float64).sum() + tl[:, 0].sum()
        s_cp += np.trace(r["out_diag"].astype(np.float64)) + tl[:, 1].sum()
        s_c += r["out_cc"].astype(np.float64).sum() + tl[:, 2].sum()
    return s_p, s_cp, s_c, res


def kernel(x, targets):
    s_p, s_cp, s_c, _ = run_device(x, targets)
    sum0 = (s_p + s_cp) / 2.0
    sum1 = (s_p - s_cp) / 2.0
    n1 = (float(N) - s_c) / 2.0
    n0 = float(N) - n1
    p = sum0 / n0 if n0 > 0 else 0.0
    r = sum1 / n1 if n1 > 0 else 0.0
    return np.array(p + r, dtype=np.float32)
